# revision 6
# baseline (speedup 1.0000x reference)
"""EntAttentionLayer on 8 TRN2 NeuronCores.

Sharding: pure sequence-parallel, no collectives. Core c handles batch
b = c//4 and query rows [qc*512, qc*512+512), qc = c%4. Each core
computes K/V for its batch's FULL sequence (redundant x4, avoids
collectives), its own 512 queries, and the whole per-row pipeline
(SA -> CA over tags -> FFN) for its rows.

Key device-side tricks:
- fp32r matmuls everywhere (full PE rate for N>=256, ~tf32 precision).
- Scores computed transposed S^T[k, q] so ctx needs no transpose of E.
- Band mask: keys are ROTATED per-core on the host (softmax is
  permutation-invariant over keys) so the |q-k|<=50 band lands in key
  chunks 0..4 for every core -> uniform SPMD instruction stream; the
  mask itself is per-core input data.
- Softmax denominator: V is augmented with a ones column per head
  (65 cols/head) so each ctx matmul emits [64 ctx rows + 1 denom row].
- 1/sqrt(var) for LN via exp(-0.5*ln(var+eps)) to stay in the
  natural_log_exp ACT table set (avoids table thrash).
- Attention q/k scale 1/8 folded into Wq on the host.
"""
import sys
sys.path.insert(0, "/opt/trn_rl_repo")
import numpy as np
import concourse.bass as bass
import concourse.mybir as mybir
import concourse.tile as tile
from concourse import bacc
from concourse import bass_utils

B, S, D, H, T, RAD = 2, 2048, 768, 12, 64, 50
DH = D // H          # 64
F = 4 * D            # 3072
SQ = S // 4          # 512 query rows per core
P = 128
NC = 8
HA = 65              # aug head width (64 ctx dims + 1 denom)
DA = H * HA          # 780
F32 = mybir.dt.float32
F32R = mybir.dt.float32r
AF = mybir.ActivationFunctionType
ALU = mybir.AluOpType
EPS = 1e-12

_CACHED_NC = None


def _layernorm(nc, lnp, r_ap, eps_sb, g_bc, b_bc, out_ap):
    """LN over free dim (768) of r_ap [P, D] -> out_ap (f32)."""
    st = lnp.tile([P, 3, 6], F32, name="ln_st")
    for g in range(3):
        nc.vector.bn_stats(st[:, g, :], r_ap[:, g * 256:(g + 1) * 256])
    mv = lnp.tile([P, 2], F32, name="ln_mv")
    nc.vector.bn_aggr(mv[:], st[:])
    lnv = lnp.tile([P, 1], F32, name="ln_lnv")
    nc.scalar.activation(lnv[:], mv[:, 1:2], AF.Ln, bias=eps_sb[:, 0:1])
    rstd = lnp.tile([P, 1], F32, name="ln_rstd")
    nc.scalar.activation(rstd[:], lnv[:], AF.Exp, bias=0.0, scale=-0.5)
    t = lnp.tile([P, D], F32, name="ln_t")
    nc.vector.tensor_scalar(out=t[:], in0=r_ap, scalar1=mv[:, 0:1],
                            scalar2=rstd[:, 0:1], op0=ALU.subtract,
                            op1=ALU.mult)
    nc.vector.tensor_mul(t[:], t[:], g_bc)
    nc.vector.tensor_add(out_ap, t[:], b_bc)


def build_kernel():
    nc = bacc.Bacc("TRN2", target_bir_lowering=False, debug=False,
                   num_devices=NC)

    def din(name, shape, dt=F32R):
        return nc.dram_tensor(name, shape, dt, kind="ExternalInput").ap()

    # --- per-core inputs ---
    xT = din("xT", [D, S])                        # rotated hidden^T
    xres = din("xres", [SQ, D], F32)              # X rows + sa_bo
    m5 = din("mask5", [P, 5, SQ], F32)            # band mask, rotated coords
    wq = din("wq", [D, D]);  bq = din("bq", [D], F32)      # pre-scaled 1/8
    wk = din("wk", [D, D]);  bk = din("bk", [D], F32)
    wv = din("wv", [D, DA]); bv_bc = din("bv_bc", [P, DA], F32)
    wo = din("wo", [D, D])
    tagT = din("tagT", [D, T])
    cwq = din("cwq", [D, D]); cbq = din("cbq", [D], F32)   # pre-scaled 1/8
    cwk = din("cwk", [D, D]); cbk = din("cbk", [D], F32)
    cwv = din("cwv", [D, DA]); cbv_bc = din("cbv_bc", [T, DA], F32)
    cwo = din("cwo", [D, D]); cbo_bc = din("cbo_bc", [P, D], F32)
    w1 = din("w1", [D, F]); b1p = din("b1p", [P, F // P], F32)
    w2 = din("w2", [F, D]); b2_bc = din("b2_bc", [P, D], F32)
    g1_bc = din("g1_bc", [P, D], F32); b1l_bc = din("b1l_bc", [P, D], F32)
    g2_bc = din("g2_bc", [P, D], F32); b2l_bc = din("b2l_bc", [P, D], F32)
    g3_bc = din("g3_bc", [P, D], F32); b3l_bc = din("b3l_bc", [P, D], F32)
    ident = din("ident", [P, P], F32)
    out = nc.dram_tensor("out", [SQ, D], F32, kind="ExternalOutput").ap()

    # internal DRAM scratch
    den_dr = nc.dram_tensor("den_dr", [H, SQ], F32).ap()
    rden_dr = nc.dram_tensor("rden_dr", [H, SQ], F32).ap()
    cden_dr = nc.dram_tensor("cden_dr", [H, SQ], F32).ap()
    crden_dr = nc.dram_tensor("crden_dr", [H, SQ], F32).ap()
    z_dr = nc.dram_tensor("z_dr", [SQ, D], F32).ap()
    zT_dr = nc.dram_tensor("zT_dr", [D, SQ], F32R).ap()

    with tile.TileContext(nc) as tc:
      with tc.tile_pool(name="consts", bufs=1) as consts:
        eps_sb = consts.tile([P, 1], F32, name="eps")
        nc.vector.memset(eps_sb[:], EPS)
        bq_sb = consts.tile([P, 6], F32, name="bq")
        nc.sync.dma_start(bq_sb[:], bq.rearrange("(c p) -> p c", p=P))
        bk_sb = consts.tile([P, 6], F32, name="bk")
        nc.sync.dma_start(bk_sb[:], bk.rearrange("(c p) -> p c", p=P))
        cbq_sb = consts.tile([P, 6], F32, name="cbq")
        nc.sync.dma_start(cbq_sb[:], cbq.rearrange("(c p) -> p c", p=P))
        cbk_sb = consts.tile([P, 6], F32, name="cbk")
        nc.sync.dma_start(cbk_sb[:], cbk.rearrange("(c p) -> p c", p=P))

        # ======== stages 1-4 under the att pool; stage 5 after it ========
        with tc.tile_pool(name="att", bufs=1) as att:
            ctxU = att.tile([64, H, SQ], F32R, name="ctxU")
            kca_sb = att.tile([P, 6, T], F32R, name="kca")
            vca_sb = att.tile([T, DA], F32R, name="vca")
            ident_sb = att.tile([P, P], F32, name="ident")
            nc.sync.dma_start(ident_sb[:], ident)

            # ---------- Stage 1: tag-table K/V ----------
            with tc.tile_pool(name="caw", bufs=1) as caw, \
                 tc.tile_pool(name="ps1", bufs=2, space="PSUM") as ps1:
                cbv_sb = caw.tile([T, DA], F32, name="cbv")
                nc.sync.dma_start(cbv_sb[:], cbv_bc)
                tagT_sb = caw.tile([P, 6, T], F32R, name="tagT")
                nc.sync.dma_start(tagT_sb[:],
                                  tagT.rearrange("(c p) t -> p c t", p=P))
                cwk_t = caw.tile([P, 6, D], F32R, name="cwk_t")
                nc.sync.dma_start(cwk_t[:],
                                  cwk.rearrange("(c p) e -> p c e", p=P))
                cwv_t = caw.tile([P, 6, DA], F32R, name="cwv_t")
                nc.sync.dma_start(cwv_t[:],
                                  cwv.rearrange("(c p) e -> p c e", p=P))
                for dc in range(6):
                    ps = ps1.tile([P, T], F32, name="ps_kca")
                    for cc in range(6):
                        nc.tensor.matmul(ps[:],
                                         cwk_t[:, cc, dc * P:(dc + 1) * P],
                                         tagT_sb[:, cc, :],
                                         start=(cc == 0), stop=(cc == 5))
                    nc.vector.tensor_scalar(out=kca_sb[:, dc, :], in0=ps[:],
                                            scalar1=cbk_sb[:, dc:dc + 1],
                                            scalar2=None, op0=ALU.add)
                psa = ps1.tile([T, 512], F32, name="ps_vca_a")
                psb = ps1.tile([T, DA - 512], F32, name="ps_vca_b")
                for cc in range(6):
                    nc.tensor.matmul(psa[:], tagT_sb[:, cc, :],
                                     cwv_t[:, cc, 0:512],
                                     start=(cc == 0), stop=(cc == 5))
                    nc.tensor.matmul(psb[:], tagT_sb[:, cc, :],
                                     cwv_t[:, cc, 512:DA],
                                     start=(cc == 0), stop=(cc == 5))
                nc.vector.tensor_add(vca_sb[:, 0:512], psa[:], cbv_sb[:, 0:512])
                nc.vector.tensor_add(vca_sb[:, 512:DA], psb[:],
                                     cbv_sb[:, 512:DA])

            # ---------- Stage 2: self-attention, two halves ----------
            HH = DA // 2  # 390 aug cols per half
            with tc.tile_pool(name="xt", bufs=1) as xtp, \
                 tc.tile_pool(name="m5p", bufs=1) as m5p, \
                 tc.tile_pool(name="kv", bufs=1) as kvp, \
                 tc.tile_pool(name="wst", bufs=2) as wst, \
                 tc.tile_pool(name="ep", bufs=4) as epool, \
                 tc.tile_pool(name="mt", bufs=2) as mtp, \
                 tc.tile_pool(name="dup", bufs=2) as dup:
                xT_sb = xtp.tile([P, 6, S], F32R, name="xT")
                nc.sync.dma_start(xT_sb[:],
                                  xT.rearrange("(c p) s -> p c s", p=P))
                bv_sb = xtp.tile([P, DA], F32, name="bv")
                nc.sync.dma_start(bv_sb[:], bv_bc)
                m5_sb = m5p.tile([P, 5, SQ], F32, name="m5")
                nc.sync.dma_start(m5_sb[:], m5)

                for half in range(2):
                    kT_sb = kvp.tile([P, 3, S], F32R, name="kT")
                    v_sb = kvp.tile([P, 16, HH], F32R, name="v")
                    qT_sb = kvp.tile([P, 3, SQ], F32R, name="qT")

                    with tc.tile_pool(name="pj", bufs=3, space="PSUM") as pj:
                        wv_t = wst.tile([P, 6, HH], F32R, name="w_t")
                        nc.sync.dma_start(
                            wv_t[:],
                            wv.rearrange("(c p) e -> p c e", p=P)[
                                :, :, half * HH:(half + 1) * HH])
                        for sc in range(16):
                            ps = pj.tile([P, HH], F32, name="ps_pj")
                            for cc in range(6):
                                nc.tensor.matmul(
                                    ps[:], xT_sb[:, cc, sc * P:(sc + 1) * P],
                                    wv_t[:, cc, :],
                                    start=(cc == 0), stop=(cc == 5))
                            nc.vector.tensor_add(
                                v_sb[:, sc, :], ps[:],
                                bv_sb[:, half * HH:(half + 1) * HH])
                        wk_t = wst.tile([P, 6, 3 * P], F32R, name="w_t")
                        nc.sync.dma_start(
                            wk_t[:],
                            wk.rearrange("(c p) e -> p c e", p=P)[
                                :, :, half * 384:(half + 1) * 384])
                        for dcl in range(3):
                            dc = half * 3 + dcl
                            for scc in range(4):
                                ps = pj.tile([P, 512], F32, name="ps_pj")
                                for cc in range(6):
                                    nc.tensor.matmul(
                                        ps[:],
                                        wk_t[:, cc, dcl * P:(dcl + 1) * P],
                                        xT_sb[:, cc, scc * 512:(scc + 1) * 512],
                                        start=(cc == 0), stop=(cc == 5))
                                nc.vector.tensor_scalar(
                                    out=kT_sb[:, dcl, scc * 512:(scc + 1) * 512],
                                    in0=ps[:], scalar1=bk_sb[:, dc:dc + 1],
                                    scalar2=None, op0=ALU.add)
                        wq_t = wst.tile([P, 6, 3 * P], F32R, name="w_t")
                        nc.sync.dma_start(
                            wq_t[:],
                            wq.rearrange("(c p) e -> p c e", p=P)[
                                :, :, half * 384:(half + 1) * 384])
                        for dcl in range(3):
                            dc = half * 3 + dcl
                            ps = pj.tile([P, 512], F32, name="ps_pj")
                            for cc in range(6):
                                nc.tensor.matmul(
                                    ps[:], wq_t[:, cc, dcl * P:(dcl + 1) * P],
                                    xT_sb[:, cc, 64:64 + SQ],
                                    start=(cc == 0), stop=(cc == 5))
                            nc.vector.tensor_scalar(
                                out=qT_sb[:, dcl, :], in0=ps[:],
                                scalar1=bq_sb[:, dc:dc + 1],
                                scalar2=None, op0=ALU.add)

                    with tc.tile_pool(name="scs", bufs=4, space="PSUM") as scs, \
                         tc.tile_pool(name="cxs", bufs=2, space="PSUM") as cxs:
                        for pl in range(3):
                            pg = half * 3 + pl
                            ha, hb = 2 * pg, 2 * pg + 1
                            la, lb = 2 * pl, 2 * pl + 1
                            ctxA = cxs.tile([HA, SQ], F32, name="ctx")
                            ctxB = cxs.tile([HA, SQ], F32, name="ctx")
                            for kc in range(16):
                                sA = scs.tile([P, SQ], F32, name="s")
                                sB = scs.tile([P, SQ], F32, name="s")
                                nc.tensor.matmul(
                                    sA[:], kT_sb[0:64, pl, kc * P:(kc + 1) * P],
                                    qT_sb[0:64, pl, :], start=True, stop=True)
                                nc.tensor.matmul(
                                    sB[:], kT_sb[64:P, pl, kc * P:(kc + 1) * P],
                                    qT_sb[64:P, pl, :], start=True, stop=True)
                                eA = epool.tile([P, SQ], F32R, name="e")
                                eB = epool.tile([P, SQ], F32R, name="e")
                                if kc < 5:
                                    mA = mtp.tile([P, SQ], F32, name="mskd")
                                    nc.vector.tensor_add(mA[:], sA[:],
                                                         m5_sb[:, kc, :])
                                    nc.scalar.activation(eA[:], mA[:], AF.Exp)
                                    mB = mtp.tile([P, SQ], F32, name="mskd")
                                    nc.vector.tensor_add(mB[:], sB[:],
                                                         m5_sb[:, kc, :])
                                    nc.scalar.activation(eB[:], mB[:], AF.Exp)
                                else:
                                    nc.scalar.activation(eA[:], sA[:], AF.Exp)
                                    nc.scalar.activation(eB[:], sB[:], AF.Exp)
                                nc.tensor.matmul(
                                    ctxA[:], v_sb[:, kc, la * HA:(la + 1) * HA],
                                    eA[:], start=(kc == 0), stop=(kc == 15))
                                nc.tensor.matmul(
                                    ctxB[:], v_sb[:, kc, lb * HA:(lb + 1) * HA],
                                    eB[:], start=(kc == 0), stop=(kc == 15))
                            for hh, cx in ((ha, ctxA), (hb, ctxB)):
                                nc.vector.tensor_copy(ctxU[:, hh, :], cx[0:64, :])
                                du = dup.tile([HA, SQ], F32, name="du")
                                nc.vector.tensor_copy(du[64:65, :], cx[64:65, :])
                                nc.sync.dma_start(den_dr[hh:hh + 1, :],
                                                  du[64:65, :])

            # ---------- Stage 3: normalize, SA out-proj, LN1, A^T ----------
            with tc.tile_pool(name="p34", bufs=1) as p34:
                a_sb = p34.tile([P, 4, D], F32, name="a_sb")
                aT_sb = p34.tile([P, 6, SQ], F32R, name="aT")
                with tc.tile_pool(name="st3", bufs=1) as st3, \
                     tc.tile_pool(name="lnp", bufs=3) as lnp, \
                     tc.tile_pool(name="rbp", bufs=3) as rbp, \
                     tc.tile_pool(name="pso", bufs=2, space="PSUM") as pso, \
                     tc.tile_pool(name="pst", bufs=2, space="PSUM") as pst:
                    den_sb = st3.tile([H, SQ], F32, name="den")
                    nc.sync.dma_start(den_sb[:], den_dr)
                    rden_sb = st3.tile([H, SQ], F32, name="rden")
                    scr = st3.tile([H, SQ], F32, name="scr")
                    nc.vector.reciprocal_approx_accurate(rden_sb[:], den_sb[:],
                                                         scr[:])
                    nc.sync.dma_start(rden_dr, rden_sb[:])
                    for h in range(H):
                        rb = rbp.tile([64, SQ], F32, name="rb")
                        nc.gpsimd.dma_start(
                            out=rb[:],
                            in_=rden_dr[h:h + 1, :].to_broadcast((64, SQ)))
                        nc.vector.tensor_mul(ctxU[:, h, :],
                                             ctxU[:, h, :].bitcast(F32), rb[:])

                    xres_sb = st3.tile([P, 4, D], F32, name="xres")
                    nc.sync.dma_start(xres_sb[:],
                                      xres.rearrange("(q p) e -> p q e", p=P))
                    g1_sb = st3.tile([P, D], F32, name="g1")
                    nc.sync.dma_start(g1_sb[:], g1_bc)
                    b1l_sb = st3.tile([P, D], F32, name="b1l")
                    nc.sync.dma_start(b1l_sb[:], b1l_bc)
                    wo_t = st3.tile([64, H, D], F32R, name="wo_t")
                    nc.sync.dma_start(wo_t[:],
                                      wo.rearrange("(h p) e -> p h e", p=64))
                    for qt in range(4):
                        po = pso.tile([P, D], F32, name="po")
                        for h in range(H):
                            nc.tensor.matmul(
                                po[:, 0:512],
                                ctxU[:, h, qt * P:(qt + 1) * P],
                                wo_t[:, h, 0:512],
                                start=(h == 0), stop=(h == H - 1))
                            nc.tensor.matmul(
                                po[:, 512:D],
                                ctxU[:, h, qt * P:(qt + 1) * P],
                                wo_t[:, h, 512:D],
                                start=(h == 0), stop=(h == H - 1))
                        r = lnp.tile([P, D], F32, name="r")
                        nc.vector.tensor_add(r[:], xres_sb[:, qt, :], po[:])
                        _layernorm(nc, lnp, r[:], eps_sb, g1_sb[:], b1l_sb[:],
                                   a_sb[:, qt, :])
                    for qt in range(4):
                        for ec in range(6):
                            pt = pst.tile([P, P], F32, name="pt")
                            nc.tensor.transpose(
                                pt[:], a_sb[:, qt, ec * P:(ec + 1) * P],
                                ident_sb[:])
                            nc.vector.tensor_copy(
                                aT_sb[:, ec, qt * P:(qt + 1) * P], pt[:])

                # ---------- Stage 4: cross-attention, LN2, Z^T ----------
                with tc.tile_pool(name="st4", bufs=1) as st4, \
                     tc.tile_pool(name="cqs", bufs=2) as cqs, \
                     tc.tile_pool(name="lnp4", bufs=3) as lnp4, \
                     tc.tile_pool(name="rbp4", bufs=3) as rbp4, \
                     tc.tile_pool(name="ep4", bufs=4) as ep4, \
                     tc.tile_pool(name="dup4", bufs=2) as dup4, \
                     tc.tile_pool(name="ztc", bufs=3) as ztc:
                    qcaT_sb = st4.tile([P, 6, SQ], F32R, name="qcaT")
                    with tc.tile_pool(name="ps4", bufs=3, space="PSUM") as ps4, \
                         tc.tile_pool(name="cx4", bufs=2, space="PSUM") as cx4:
                        for dc in range(6):
                            cwq_t = cqs.tile([P, 6, P], F32R, name="cwq_t")
                            nc.sync.dma_start(
                                cwq_t[:],
                                cwq.rearrange("(c p) e -> p c e", p=P)[
                                    :, :, dc * P:(dc + 1) * P])
                            ps = ps4.tile([P, 512], F32, name="ps4t")
                            for cc in range(6):
                                nc.tensor.matmul(ps[:], cwq_t[:, cc, :],
                                                 aT_sb[:, cc, :],
                                                 start=(cc == 0), stop=(cc == 5))
                            nc.vector.tensor_scalar(
                                out=qcaT_sb[:, dc, :], in0=ps[:],
                                scalar1=cbq_sb[:, dc:dc + 1],
                                scalar2=None, op0=ALU.add)
                        for pg in range(6):
                            ha, hb = 2 * pg, 2 * pg + 1
                            sA = ps4.tile([T, SQ], F32, name="ps4t")
                            sB = ps4.tile([T, SQ], F32, name="ps4t")
                            nc.tensor.matmul(sA[:], kca_sb[0:64, pg, :],
                                             qcaT_sb[0:64, pg, :],
                                             start=True, stop=True)
                            nc.tensor.matmul(sB[:], kca_sb[64:P, pg, :],
                                             qcaT_sb[64:P, pg, :],
                                             start=True, stop=True)
                            for hh, sx in ((ha, sA), (hb, sB)):
                                ex = ep4.tile([T, SQ], F32R, name="e4")
                                nc.scalar.activation(ex[:], sx[:], AF.Exp)
                                cx = cx4.tile([HA, SQ], F32, name="cx4t")
                                nc.tensor.matmul(
                                    cx[:], vca_sb[:, hh * HA:(hh + 1) * HA],
                                    ex[:], start=True, stop=True)
                                nc.vector.tensor_copy(ctxU[:, hh, :], cx[0:64, :])
                                du = dup4.tile([HA, SQ], F32, name="du4")
                                nc.vector.tensor_copy(du[64:65, :], cx[64:65, :])
                                nc.sync.dma_start(cden_dr[hh:hh + 1, :],
                                                  du[64:65, :])

                    den_sb = st4.tile([H, SQ], F32, name="cden")
                    nc.sync.dma_start(den_sb[:], cden_dr)
                    rden_sb = st4.tile([H, SQ], F32, name="crden")
                    scr = st4.tile([H, SQ], F32, name="cscr")
                    nc.vector.reciprocal_approx_accurate(rden_sb[:], den_sb[:],
                                                         scr[:])
                    nc.sync.dma_start(crden_dr, rden_sb[:])
                    for h in range(H):
                        rb = rbp4.tile([64, SQ], F32, name="rb4")
                        nc.gpsimd.dma_start(
                            out=rb[:],
                            in_=crden_dr[h:h + 1, :].to_broadcast((64, SQ)))
                        nc.vector.tensor_mul(ctxU[:, h, :],
                                             ctxU[:, h, :].bitcast(F32), rb[:])

                    cbo_sb = st4.tile([P, D], F32, name="cbo")
                    nc.sync.dma_start(cbo_sb[:], cbo_bc)
                    g2_sb = st4.tile([P, D], F32, name="g2")
                    nc.sync.dma_start(g2_sb[:], g2_bc)
                    b2l_sb = st4.tile([P, D], F32, name="b2l")
                    nc.sync.dma_start(b2l_sb[:], b2l_bc)
                    z_sb = st4.tile([P, 4, D], F32, name="z_sb")
                    with tc.tile_pool(name="pso4", bufs=1, space="PSUM") as pso4:
                        pos = [pso4.tile([P, D], F32, name=f"po4_{qt}")
                               for qt in range(4)]
                        for h in range(H):
                            cwo_t = cqs.tile([64, D], F32R, name="cwo_t")
                            nc.sync.dma_start(
                                cwo_t[:],
                                cwo.rearrange("(h p) e -> p h e", p=64)[:, h, :])
                            for qt in range(4):
                                nc.tensor.matmul(
                                    pos[qt][:, 0:512],
                                    ctxU[:, h, qt * P:(qt + 1) * P],
                                    cwo_t[:, 0:512],
                                    start=(h == 0), stop=(h == H - 1))
                                nc.tensor.matmul(
                                    pos[qt][:, 512:D],
                                    ctxU[:, h, qt * P:(qt + 1) * P],
                                    cwo_t[:, 512:D],
                                    start=(h == 0), stop=(h == H - 1))
                        for qt in range(4):
                            r = lnp4.tile([P, D], F32, name="r4")
                            nc.vector.tensor_add(r[:], a_sb[:, qt, :],
                                                 pos[qt][:])
                            nc.vector.tensor_add(r[:], r[:], cbo_sb[:])
                            _layernorm(nc, lnp4, r[:], eps_sb, g2_sb[:],
                                       b2l_sb[:], z_sb[:, qt, :])
                    nc.sync.dma_start(z_dr.rearrange("(q p) e -> p q e", p=P),
                                      z_sb[:])
                    with tc.tile_pool(name="pst4", bufs=2,
                                      space="PSUM") as pst4:
                        for qt in range(4):
                            for ec in range(6):
                                pt = pst4.tile([P, P], F32, name="pt4")
                                nc.tensor.transpose(
                                    pt[:], z_sb[:, qt, ec * P:(ec + 1) * P],
                                    ident_sb[:])
                                zc = ztc.tile([P, P], F32R, name="zc")
                                nc.vector.tensor_copy(zc[:], pt[:])
                                nc.sync.dma_start(
                                    zT_dr[ec * P:(ec + 1) * P,
                                          qt * P:(qt + 1) * P], zc[:])

        # ---------- Stage 5: FFN + LN3 + output ----------
        with tc.tile_pool(name="st5", bufs=1) as st5, \
             tc.tile_pool(name="w1p", bufs=1) as w1p, \
             tc.tile_pool(name="lnp5", bufs=3) as lnp5, \
             tc.tile_pool(name="w2p", bufs=3) as w2p:
            z_sb = st5.tile([P, 4, D], F32, name="z5")
            nc.sync.dma_start(z_sb[:], z_dr.rearrange("(q p) e -> p q e", p=P))
            zT_sb = st5.tile([P, 6, SQ], F32R, name="zT5")
            nc.sync.dma_start(zT_sb[:],
                              zT_dr.rearrange("(c p) s -> p c s", p=P))
            b1p_sb = st5.tile([P, F // P, 1], F32, name="b1p")
            nc.sync.dma_start(b1p_sb[:], b1p[:, :, None])
            ig_sb = st5.tile([P, F // P, SQ], F32R, name="ig")
            with tc.tile_pool(name="ps5", bufs=3, space="PSUM") as ps5:
                for h5 in range(2):
                    w1_t = w1p.tile([P, 6, F // 2], F32R, name="w1_t")
                    nc.sync.dma_start(
                        w1_t[:],
                        w1.rearrange("(c p) e -> p c e", p=P)[
                            :, :, h5 * (F // 2):(h5 + 1) * (F // 2)])
                    for i in range(12):
                        fc = h5 * 12 + i
                        ps = ps5.tile([P, SQ], F32, name="ps5t")
                        for cc in range(6):
                            nc.tensor.matmul(ps[:],
                                             w1_t[:, cc, i * P:(i + 1) * P],
                                             zT_sb[:, cc, :],
                                             start=(cc == 0), stop=(cc == 5))
                        nc.scalar.activation(ig_sb[:, fc, :], ps[:], AF.Gelu,
                                             bias=b1p_sb[:, fc, 0:1])

            g3_sb = st5.tile([P, D], F32, name="g3")
            nc.sync.dma_start(g3_sb[:], g3_bc)
            b3l_sb = st5.tile([P, D], F32, name="b3l")
            nc.sync.dma_start(b3l_sb[:], b3l_bc)
            b2r_sb = st5.tile([P, D], F32, name="b2r")
            nc.sync.dma_start(b2r_sb[:], b2_bc)

            with tc.tile_pool(name="pso5", bufs=1, space="PSUM") as pso5:
                pos = [pso5.tile([P, D], F32, name=f"po5_{qt}")
                       for qt in range(4)]
                for fc in range(F // P):
                    w2_t = w2p.tile([P, D], F32R, name="w2_t")
                    nc.sync.dma_start(w2_t[:], w2[fc * P:(fc + 1) * P, :])
                    for qt in range(4):
                        nc.tensor.matmul(pos[qt][:, 0:512],
                                         ig_sb[:, fc, qt * P:(qt + 1) * P],
                                         w2_t[:, 0:512],
                                         start=(fc == 0), stop=(fc == F // P - 1))
                        nc.tensor.matmul(pos[qt][:, 512:D],
                                         ig_sb[:, fc, qt * P:(qt + 1) * P],
                                         w2_t[:, 512:D],
                                         start=(fc == 0), stop=(fc == F // P - 1))
                for qt in range(4):
                    r = lnp5.tile([P, D], F32, name="r5")
                    nc.vector.tensor_add(r[:], z_sb[:, qt, :], pos[qt][:])
                    nc.vector.tensor_add(r[:], r[:], b2r_sb[:])
                    o_sb = lnp5.tile([P, D], F32, name="o5")
                    _layernorm(nc, lnp5, r[:], eps_sb, g3_sb[:], b3l_sb[:],
                               o_sb[:])
                    nc.sync.dma_start(out[qt * P:(qt + 1) * P, :], o_sb[:])

    nc.compile()
    return nc


def _prep_shared(inp):
    """Host-side shared (core-independent) arrays."""
    f32 = np.float32
    sh = {}
    sh["wq"] = np.ascontiguousarray(inp["sa_wq"] * 0.125)
    sh["bq"] = np.ascontiguousarray(inp["sa_bq"] * 0.125)
    sh["wk"] = np.ascontiguousarray(inp["sa_wk"])
    sh["bk"] = np.ascontiguousarray(inp["sa_bk"])

    def aug(wv, bv):
        wva = np.zeros((D, DA), f32)
        bva = np.zeros((DA,), f32)
        for h in range(H):
            wva[:, h * HA:h * HA + DH] = wv[:, h * DH:(h + 1) * DH]
            bva[h * HA:h * HA + DH] = bv[h * DH:(h + 1) * DH]
            bva[h * HA + DH] = 1.0
        return wva, bva

    wva, bva = aug(inp["sa_wv"], inp["sa_bv"])
    sh["wv"] = wva
    sh["bv_bc"] = np.ascontiguousarray(np.broadcast_to(bva, (P, DA)))
    sh["wo"] = np.ascontiguousarray(inp["sa_wo"])
    sh["tagT"] = np.ascontiguousarray(inp["tag_emb"].T)
    sh["cwq"] = np.ascontiguousarray(inp["ca_wq"] * 0.125)
    sh["cbq"] = np.ascontiguousarray(inp["ca_bq"] * 0.125)
    sh["cwk"] = np.ascontiguousarray(inp["ca_wk"])
    sh["cbk"] = np.ascontiguousarray(inp["ca_bk"])
    cwva, cbva = aug(inp["ca_wv"], inp["ca_bv"])
    sh["cwv"] = cwva
    sh["cbv_bc"] = np.ascontiguousarray(np.broadcast_to(cbva, (T, DA)))
    sh["cwo"] = np.ascontiguousarray(inp["ca_wo"])
    sh["cbo_bc"] = np.ascontiguousarray(np.broadcast_to(inp["ca_bo"], (P, D)))
    sh["w1"] = np.ascontiguousarray(inp["ff_w1"])
    sh["b1p"] = np.ascontiguousarray(inp["ff_b1"].reshape(F // P, P).T)
    sh["w2"] = np.ascontiguousarray(inp["ff_w2"])
    sh["b2_bc"] = np.ascontiguousarray(np.broadcast_to(inp["ff_b2"], (P, D)))
    sh["g1_bc"] = np.ascontiguousarray(np.broadcast_to(inp["sa_ln_g"], (P, D)))
    sh["b1l_bc"] = np.ascontiguousarray(np.broadcast_to(inp["sa_ln_b"], (P, D)))
    sh["g2_bc"] = np.ascontiguousarray(np.broadcast_to(inp["ca_ln_g"], (P, D)))
    sh["b2l_bc"] = np.ascontiguousarray(np.broadcast_to(inp["ca_ln_b"], (P, D)))
    sh["g3_bc"] = np.ascontiguousarray(np.broadcast_to(inp["ff_ln_g"], (P, D)))
    sh["b3l_bc"] = np.ascontiguousarray(np.broadcast_to(inp["ff_ln_b"], (P, D)))
    sh["ident"] = np.eye(P, dtype=f32)
    return sh


def _mask5_for(qc):
    q0 = qc * SQ
    pos = np.arange(5 * P)
    s_true = (pos - 64 + q0) % S
    u = np.arange(SQ)
    band = (np.abs((q0 + u)[None, :] - s_true[:, None]) <= RAD).astype(np.float32)
    return np.ascontiguousarray(band.reshape(5, P, SQ).transpose(1, 0, 2))


def _make_in_maps(inp):
    sh = _prep_shared(inp)
    masks = [_mask5_for(qc) for qc in range(4)]
    hs = inp["hidden_states"]
    in_maps = []
    for c in range(NC):
        b, qc = c // 4, c % 4
        q0 = qc * SQ
        xTb = np.ascontiguousarray(hs[b].T)
        m = dict(sh)
        m["xT"] = np.ascontiguousarray(np.roll(xTb, 64 - q0, axis=1))
        m["xres"] = np.ascontiguousarray(hs[b, q0:q0 + SQ] + inp["sa_bo"])
        m["mask5"] = masks[qc]
        in_maps.append(m)
    return in_maps


def kernel(**inputs):
    global _CACHED_NC
    inp = {k: np.asarray(v, dtype=np.float32) for k, v in inputs.items()}
    if _CACHED_NC is None:
        _CACHED_NC = build_kernel()
    nc = _CACHED_NC

    in_maps = _make_in_maps(inp)
    res = bass_utils.run_bass_kernel_spmd(nc, in_maps, core_ids=list(range(NC)))
    out = np.empty((B, S, D), np.float32)
    for c in range(NC):
        b, qc = c // 4, c % 4
        out[b, qc * SQ:(qc + 1) * SQ] = res.results[c]["out"]
    return out
